# revision 26
# baseline (speedup 1.0000x reference)
"""Trainium2 Bass kernel for the ContextComputer GNN message-passing module.

Computation (per batch row b):
    W1, W2 = W[:D], W[D:]
    u_j    = memory_j * mask_j                       # [N, D]
    a_i    = memory_i @ W1 + bias                    # [N, D]
    c_j    = mask_j * (memory_j @ W2)                # [N, D]
    ctx_i  = sum_{j != i} sigmoid(a_i + c_j) * u_j

Sharding: pure data parallel over batch B across the 8 NeuronCores
(B=8192 -> 1024 rows per core); W/b replicated.

Per-core layout: batch rows on the 128 SBUF partitions, features on the
free axis. v2 pairwise stage (off-diagonal pairs only, 30 per row-tile):
  - memory loads fp32, cast-stored bf16 to a DRAM scratch (SWDGE cast
    DMA); hardware transpose-DMAs produce the d-on-partition lhsT tiles.
  - PE: a_i = m_i @ W1 (+bias via rank-1 ones matmul), c'_j = m_j @ W2,
    accumulating in 8 PSUM banks.
  - gpsimd drains PSUM -> SBUF bf16 (plain casts; mask NOT applied here),
    keeping ActE free for sigmoid only.
  - DVE scalar_tensor_tensor (TensorScalarPtr 4x mode) does everything
    else: t_ij = (c'_j * mask_j) + a_i, p_ij = (m_j * mask_j) * g_ij,
    and the 5-term j-sum as a 3-op paired tree.
  - Act: one sigmoid per two i ([P, 2*5*D]).
"""

import numpy as np

import concourse.bass as bass
import concourse.mybir as mybir
import concourse.tile as tile
from concourse.bass_utils import run_bass_kernel_spmd

B, N, D = 8192, 6, 512
P = 128
DC = D // P  # 4 contraction chunks of 128
NCORES = 8
BLOC = B // NCORES

F32 = mybir.dt.float32
BF16 = mybir.dt.bfloat16

_ADD = mybir.AluOpType.add
_MULT = mybir.AluOpType.mult
_SIGMOID = mybir.ActivationFunctionType.Sigmoid

# per-i strided pairing of the 5 off-diagonal blocks: s = p[in0] + p[in1]
# (two blocks each, uniform strides), then s0+s1, then + p[rem]
_PAIRS = {
    0: (slice(1, 4, 2), slice(2, 5, 2), 5),
    1: (slice(0, 4, 3), slice(2, 5, 2), 5),
    2: (slice(0, 4, 3), slice(1, 5, 3), 5),
    3: (slice(0, 3, 2), slice(1, 5, 3), 5),
    4: (slice(0, 3, 2), slice(1, 4, 2), 5),
    5: (slice(0, 3, 2), slice(1, 4, 2), 4),
}

_nc_cache = {}


def _split_excess_waits(nc, max_waits=1):
    """The pinned walrus build only supports one sync-wait slot per
    instruction; hoist extra Tile-emitted waits onto standalone
    same-engine EventSemaphore instructions (NX dispatcher-level waits,
    so ordering semantics are preserved)."""
    f = nc.m.functions[0]
    for blk in f.blocks:
        new = []
        for ins in blk.instructions:
            si = getattr(ins, "sync_info", None)
            eng = getattr(ins, "engine", None)
            if si is not None and si.on_wait and len(si.on_wait) > max_waits and eng is not None:
                waits = list(si.on_wait)
                extra, keep = waits[:-max_waits], waits[-max_waits:]
                for k, w in enumerate(extra):
                    new.append(
                        mybir.InstEventSemaphore(
                            name=f"{ins.name}_xw{k}",
                            opcode="EventSemaphore",
                            engine=eng,
                            ins=[],
                            outs=[],
                            sync_info=mybir.SyncInfo(on_wait=[w], on_update=[]),
                        )
                    )
                si.on_wait = keep
            new.append(ins)
        blk.instructions[:] = new


def build(bloc=BLOC, split_waits=True, copy_eng="act", ctx_dtype="bf16"):
    nbt = bloc // P
    nc = bass.Bass(num_swdge_queues=4)
    mem = nc.declare_dram_parameter("memory", [bloc, N, D], F32, isOutput=False)
    msk = nc.declare_dram_parameter("mask", [bloc, N, 1], F32, isOutput=False)
    w_p = nc.declare_dram_parameter("W", [2 * D, D], F32, isOutput=False)
    b_p = nc.declare_dram_parameter("b", [D], F32, isOutput=False)
    out = nc.declare_dram_parameter("context", [bloc, N, D], F32, isOutput=True)
    mbf = nc.dram_tensor("mbf", [bloc, N, D], BF16)

    CTX_DT = F32 if ctx_dtype == "f32" else BF16

    with tile.TileContext(nc) as tc:
        with (
            tc.tile_pool(name="const", bufs=1) as constp,
            tc.tile_pool(name="mt", bufs=3) as mtp,
            tc.tile_pool(name="work", bufs=3) as work,
            tc.tile_pool(name="pair", bufs=2) as pairp,
            tc.tile_pool(name="acc", bufs=4) as accp,
            tc.tile_pool(name="maskp", bufs=8) as maskp,
            tc.tile_pool(name="outp", bufs=8) as outp,
            tc.tile_pool(name="psumA", bufs=2, space="PSUM") as pspa,
            tc.tile_pool(name="psumC", bufs=4, space="PSUM") as pspc,
        ):
            def load_consts():
                # W/bias load fp32 on the scalar HWDGE ring (SWDGE triggers
                # serialize ~1us each on Pool and would delay the critical
                # memory casts), then cast once on the otherwise-idle DVE.
                wt = {}
                with tc.tile_pool(name="wstage", bufs=2) as wstage:
                    for h in range(2):  # 0 -> W1, 1 -> W2
                        for dc in range(DC):
                            t32 = wstage.tile([P, D], F32, tag="w32")
                            nc.scalar.dma_start(
                                out=t32[:],
                                in_=w_p[h * D + dc * P : h * D + (dc + 1) * P, :],
                            )
                            t = constp.tile([P, D], BF16, tag=f"w{h}{dc}")
                            nc.vector.tensor_scalar_add(
                                out=t[:], in0=t32[:], scalar1=0.0
                            )
                            wt[h, dc] = t
                    b32 = wstage.tile([1, D], F32, tag="b32")
                    nc.scalar.dma_start(out=b32[:], in_=b_p[None, :])
                    bias_t = constp.tile([1, D], BF16, tag="bias")
                    nc.vector.tensor_scalar_add(
                        out=bias_t[:], in0=b32[:], scalar1=0.0
                    )
                ones_t = constp.tile([1, P], BF16, tag="ones")
                nc.vector.memset(ones_t[:], 1.0)
                return wt, bias_t, ones_t

            # ---- per-row-tile prepass, software-pipelined 2 tiles ahead of
            # compute: HBM->HBM bf16 cast (SWDGE; the only DMAs that must
            # cast), transpose-DMAs (sync HWDGE ring), natural bf16 load +
            # mask (scalar HWDGE ring, keeping Pool's stream clear for the
            # PSUM drains) ----
            mask_ts = {}
            m_alls = {}
            mt = {}

            def prepass(bt):
                bsl = slice(bt * P, (bt + 1) * P)
                nc.gpsimd.dma_start(out=mbf[bsl], in_=mem[bsl])
                for j in range(N):
                    # one xbar transpose per head: [128, 512] -> [128, DC, 128]
                    t = mtp.tile([P, DC, P], BF16, tag=f"mt{j}")
                    nc.sync.dma_start(out=t[:], in_=mbf[bsl, j, :], transpose=True)
                    for dc in range(DC):
                        mt[bt, j, dc] = t[:, dc, :]
                mask_t = maskp.tile([P, N], F32, tag="mask")
                nc.sync.dma_start(out=mask_t[:], in_=msk[bsl, :, 0])
                mask_ts[bt] = mask_t
                m_all = work.tile([P, N * D], BF16, tag="mnat")
                nc.sync.dma_start(
                    out=m_all.rearrange("p (n d) -> p n d", n=N), in_=mbf[bsl]
                )
                m_alls[bt] = m_all

            a_alls = {}
            c_alls = {}

            def matmuls(bt):
                """a/c chains + PSUM drains (ActE; GPSIMD can't read PSUM)
                for one row-tile. c first so the first pairwise op (which
                needs all c, a_0, a_1) unblocks sooner on the cold start.
                a-chains share a 2-bank PSUM tile per i-pair so one Act copy
                drains two chains (halves the per-copy init cost)."""
                mask_t = mask_ts[bt]
                a_all = work.tile([P, N * D], BF16, tag="a")
                c_all = work.tile([P, N * D], BF16, tag="c")
                a_alls[bt] = a_all
                c_alls[bt] = c_all
                for j in range(N):
                    c_ps = pspc.tile([P, D], F32, tag="cps")
                    for dc in range(DC):
                        nc.tensor.matmul(
                            out=c_ps[:],
                            lhsT=mt[bt, j, dc][:],
                            rhs=wt[1, dc][:],
                            start=(dc == 0),
                            stop=(dc == DC - 1),
                        )
                    # c_j = mask_j * (m_j @ W2): fold the mask into the drain
                    nc.scalar.mul(
                        out=c_all[:, j * D : (j + 1) * D],
                        in_=c_ps[:],
                        mul=mask_t[:, j : j + 1],
                    )
                for i0 in range(0, N, 2):
                    a_ps = pspa.tile([P, 2 * D], F32, tag="aps")
                    for il in range(2):
                        i = i0 + il
                        half = a_ps[:, il * D : (il + 1) * D]
                        for dc in range(DC):
                            nc.tensor.matmul(
                                out=half,
                                lhsT=mt[bt, i, dc][:],
                                rhs=wt[0, dc][:],
                                start=(dc == 0),
                                stop=False,
                            )
                        nc.tensor.matmul(
                            out=half,
                            lhsT=ones_t[:],
                            rhs=bias_t[:],
                            start=False,
                            stop=True,
                        )
                    nc.scalar.copy(
                        out=a_all[:, i0 * D : (i0 + 2) * D], in_=a_ps[:]
                    )

            PREFETCH = 3
            wt, bias_t, ones_t = load_consts()
            for bt in range(min(PREFETCH, nbt)):
                prepass(bt)
            matmuls(0)

            for bt in range(nbt):
                if bt + PREFETCH < nbt:
                    prepass(bt + PREFETCH)
                # pairwise(bt) is emitted BEFORE matmuls(bt+1) so the bt
                # sigmoids aren't queued behind bt+1's PSUM drains in the
                # Act stream; the emission order of the loop tail below is
                # pairwise(bt) work first, then matmuls(bt+1) appended by
                # the next statement group.
                bsl = slice(bt * P, (bt + 1) * P)
                mask_t = mask_ts[bt]
                m_all = m_alls[bt]
                a_all = a_alls[bt]
                c_all = c_alls[bt]

                # ---- u_j = mask_j * m_j (TensorScalar, 4x mode) ----
                u_all = work.tile([P, N * D], BF16, tag="u")
                for j in range(N):
                    nc.vector.tensor_scalar_mul(
                        out=u_all[:, j * D : (j + 1) * D],
                        in0=m_all[:, j * D : (j + 1) * D],
                        scalar1=mask_t[:, j : j + 1],
                    )
                # ---- pairwise sigmoid gating, two i per instruction,
                # diagonal included (cheaper than splitting) ----
                for i0 in range(0, N, 2):
                    a_b = (
                        a_all[:, i0 * D : (i0 + 2) * D]
                        .rearrange("p (i f) -> p i f", i=2)
                        .rearrange("p i (j f) -> p i j f", j=1)
                        .broadcast_to([P, 2, N, D])
                    )
                    c_b = (
                        c_all.rearrange("p (i f) -> p i f", i=1)
                        .broadcast_to([P, 2, N * D])
                        .rearrange("p i (j f) -> p i j f", j=N)
                    )
                    t_all = pairp.tile([P, 2 * N * D], BF16, tag="t")
                    nc.vector.tensor_tensor(
                        out=t_all.rearrange("p (i j f) -> p i j f", i=2, j=N),
                        in0=a_b,
                        in1=c_b,
                        op=_ADD,
                    )
                    g_all = pairp.tile([P, 2 * N * D], BF16, tag="g")
                    nc.scalar.activation(out=g_all[:], in_=t_all[:], func=_SIGMOID)
                    u_b = (
                        u_all.rearrange("p (i f) -> p i f", i=1)
                        .broadcast_to([P, 2, N * D])
                    )
                    p_all = pairp.tile([P, 2 * N * D], BF16, tag="pp")
                    nc.vector.tensor_tensor(
                        out=p_all.rearrange("p (i f) -> p i f", i=2),
                        in0=g_all.rearrange("p (i f) -> p i f", i=2),
                        in1=u_b,
                        op=_MULT,
                    )
                    for il in range(2):
                        i = i0 + il
                        pv = p_all[:, il * N * D : (il + 1) * N * D].rearrange(
                            "p (j f) -> p j f", j=N
                        )
                        s0, s1, rem = _PAIRS[i]
                        s = accp.tile([P, 2 * D], BF16, tag="s")
                        nc.vector.tensor_tensor(
                            out=s.rearrange("p (j f) -> p j f", j=2),
                            in0=pv[:, s0, :],
                            in1=pv[:, s1, :],
                            op=_ADD,
                        )
                        s2 = accp.tile([P, D], BF16, tag="s2")
                        nc.vector.tensor_add(out=s2[:], in0=s[:, :D], in1=s[:, D:])
                        ctx_t = outp.tile([P, D], CTX_DT, tag="ctx")
                        nc.vector.tensor_add(
                            out=ctx_t[:], in0=s2[:], in1=pv[:, rem, :]
                        )
                        if ctx_dtype == "f32":
                            nc.scalar.dma_start(out=out[bsl, i, :], in_=ctx_t[:])
                        else:
                            # SWDGE cast store bf16 -> fp32
                            nc.gpsimd.dma_start(out=out[bsl, i, :], in_=ctx_t[:])

                if bt + 1 < nbt:
                    matmuls(bt + 1)
    if split_waits:
        _split_excess_waits(nc)
    return nc


def get_nc(bloc=BLOC, **kw):
    key = (bloc, tuple(sorted(kw.items())))
    if key not in _nc_cache:
        _nc_cache[key] = build(bloc, **kw)
    return _nc_cache[key]


last_results = None


def kernel(**inputs):
    global last_results
    memory = np.ascontiguousarray(inputs["memory"], dtype=np.float32)
    mask = np.ascontiguousarray(inputs["mask"], dtype=np.float32)
    W = np.ascontiguousarray(inputs["W"], dtype=np.float32)
    b = np.ascontiguousarray(inputs["b"], dtype=np.float32)

    nc = get_nc()
    in_maps = [
        {
            "memory": memory[c * BLOC : (c + 1) * BLOC],
            "mask": mask[c * BLOC : (c + 1) * BLOC],
            "W": W,
            "b": b,
        }
        for c in range(NCORES)
    ]
    res = run_bass_kernel_spmd(nc, in_maps, list(range(NCORES)))
    last_results = res
    out = np.concatenate(
        [res.results[c]["context"] for c in range(NCORES)], axis=0
    )
    return out.astype(np.float32, copy=False)


# revision 29
# speedup vs baseline: 4.4964x; 4.4964x over previous
"""Trainium2 Bass kernel for the ContextComputer GNN message-passing module.

Computation (per batch row b):
    W1, W2 = W[:D], W[D:]
    u_j    = memory_j * mask_j                       # [N, D]
    a_i    = memory_i @ W1 + bias                    # [N, D]
    c_j    = mask_j * (memory_j @ W2)                # [N, D]
    ctx_i  = sum_{j != i} sigmoid(a_i + c_j) * u_j

Sharding: pure data parallel over batch B across the 8 NeuronCores
(B=8192 -> 1024 rows per core); W/b replicated.

Per-core layout: batch rows on the 128 SBUF partitions, features on the
free axis. v2 pairwise stage (off-diagonal pairs only, 30 per row-tile):
  - memory loads fp32, cast-stored bf16 to a DRAM scratch (SWDGE cast
    DMA); hardware transpose-DMAs produce the d-on-partition lhsT tiles.
  - PE: a_i = m_i @ W1 (+bias via rank-1 ones matmul), c'_j = m_j @ W2,
    accumulating in 8 PSUM banks.
  - gpsimd drains PSUM -> SBUF bf16 (plain casts; mask NOT applied here),
    keeping ActE free for sigmoid only.
  - DVE scalar_tensor_tensor (TensorScalarPtr 4x mode) does everything
    else: t_ij = (c'_j * mask_j) + a_i, p_ij = (m_j * mask_j) * g_ij,
    and the 5-term j-sum as a 3-op paired tree.
  - Act: one sigmoid per two i ([P, 2*5*D]).
"""

import numpy as np

import concourse.bass as bass
import concourse.mybir as mybir
import concourse.tile as tile
from concourse.bass_utils import run_bass_kernel_spmd

B, N, D = 8192, 6, 512
P = 128
DC = D // P  # 4 contraction chunks of 128
NCORES = 8
BLOC = B // NCORES

F32 = mybir.dt.float32
BF16 = mybir.dt.bfloat16

_ADD = mybir.AluOpType.add
_MULT = mybir.AluOpType.mult
_SIGMOID = mybir.ActivationFunctionType.Sigmoid

# per-i strided pairing of the 5 off-diagonal blocks: s = p[in0] + p[in1]
# (two blocks each, uniform strides), then s0+s1, then + p[rem]
_PAIRS = {
    0: (slice(1, 4, 2), slice(2, 5, 2), 5),
    1: (slice(0, 4, 3), slice(2, 5, 2), 5),
    2: (slice(0, 4, 3), slice(1, 5, 3), 5),
    3: (slice(0, 3, 2), slice(1, 5, 3), 5),
    4: (slice(0, 3, 2), slice(1, 4, 2), 5),
    5: (slice(0, 3, 2), slice(1, 4, 2), 4),
}

_nc_cache = {}


def _split_excess_waits(nc, max_waits=1):
    """The pinned walrus build only supports one sync-wait slot per
    instruction; hoist extra Tile-emitted waits onto standalone
    same-engine EventSemaphore instructions (NX dispatcher-level waits,
    so ordering semantics are preserved)."""
    f = nc.m.functions[0]
    for blk in f.blocks:
        new = []
        for ins in blk.instructions:
            si = getattr(ins, "sync_info", None)
            eng = getattr(ins, "engine", None)
            if si is not None and si.on_wait and len(si.on_wait) > max_waits and eng is not None:
                waits = list(si.on_wait)
                extra, keep = waits[:-max_waits], waits[-max_waits:]
                for k, w in enumerate(extra):
                    new.append(
                        mybir.InstEventSemaphore(
                            name=f"{ins.name}_xw{k}",
                            opcode="EventSemaphore",
                            engine=eng,
                            ins=[],
                            outs=[],
                            sync_info=mybir.SyncInfo(on_wait=[w], on_update=[]),
                        )
                    )
                si.on_wait = keep
            new.append(ins)
        blk.instructions[:] = new


def build(bloc=BLOC, split_waits=True, copy_eng="act", ctx_dtype="bf16"):
    nbt = bloc // P
    nc = bass.Bass(num_swdge_queues=4)
    mem = nc.declare_dram_parameter("memory", [bloc, N, D], F32, isOutput=False)
    msk = nc.declare_dram_parameter("mask", [bloc, N, 1], F32, isOutput=False)
    w_p = nc.declare_dram_parameter("W", [2 * D, D], F32, isOutput=False)
    b_p = nc.declare_dram_parameter("b", [D], F32, isOutput=False)
    CTX_DT = F32 if ctx_dtype == "f32" else BF16
    # bf16 output halves the store traffic and keeps the final DVE op in
    # 2x mode; kernel() upcasts to fp32 on the host.
    out = nc.declare_dram_parameter("context", [bloc, N, D], CTX_DT, isOutput=True)
    mbf = nc.dram_tensor("mbf", [bloc, N, D], BF16)

    with tile.TileContext(nc) as tc:
        with (
            tc.tile_pool(name="const", bufs=1) as constp,
            tc.tile_pool(name="mt", bufs=3) as mtp,
            tc.tile_pool(name="work", bufs=3) as work,
            tc.tile_pool(name="pair", bufs=2) as pairp,
            tc.tile_pool(name="acc", bufs=4) as accp,
            tc.tile_pool(name="maskp", bufs=8) as maskp,
            tc.tile_pool(name="outp", bufs=8) as outp,
            tc.tile_pool(name="psumA", bufs=2, space="PSUM") as pspa,
            tc.tile_pool(name="psumC", bufs=4, space="PSUM") as pspc,
        ):
            def load_consts():
                # W/bias load fp32 on the scalar HWDGE ring (SWDGE triggers
                # serialize ~1us each on Pool and would delay the critical
                # memory casts), then cast once on the otherwise-idle DVE.
                wt = {}
                with tc.tile_pool(name="wstage", bufs=2) as wstage:
                    for h in range(2):  # 0 -> W1, 1 -> W2
                        for dc in range(DC):
                            t32 = wstage.tile([P, D], F32, tag="w32")
                            nc.scalar.dma_start(
                                out=t32[:],
                                in_=w_p[h * D + dc * P : h * D + (dc + 1) * P, :],
                            )
                            t = constp.tile([P, D], BF16, tag=f"w{h}{dc}")
                            nc.vector.tensor_scalar_add(
                                out=t[:], in0=t32[:], scalar1=0.0
                            )
                            wt[h, dc] = t
                    b32 = wstage.tile([1, D], F32, tag="b32")
                    nc.scalar.dma_start(out=b32[:], in_=b_p[None, :])
                    bias_t = constp.tile([1, D], BF16, tag="bias")
                    nc.vector.tensor_scalar_add(
                        out=bias_t[:], in0=b32[:], scalar1=0.0
                    )
                ones_t = constp.tile([1, P], BF16, tag="ones")
                nc.vector.memset(ones_t[:], 1.0)
                return wt, bias_t, ones_t

            # ---- per-row-tile prepass, software-pipelined 2 tiles ahead of
            # compute: HBM->HBM bf16 cast (SWDGE; the only DMAs that must
            # cast), transpose-DMAs (sync HWDGE ring), natural bf16 load +
            # mask (scalar HWDGE ring, keeping Pool's stream clear for the
            # PSUM drains) ----
            mask_ts = {}
            m_alls = {}
            mt = {}

            def prepass(bt):
                bsl = slice(bt * P, (bt + 1) * P)
                nc.gpsimd.dma_start(out=mbf[bsl], in_=mem[bsl])
                for j in range(N):
                    # one xbar transpose per head: [128, 512] -> [128, DC, 128]
                    t = mtp.tile([P, DC, P], BF16, tag=f"mt{j}")
                    nc.sync.dma_start(out=t[:], in_=mbf[bsl, j, :], transpose=True)
                    for dc in range(DC):
                        mt[bt, j, dc] = t[:, dc, :]
                mask_t = maskp.tile([P, N], F32, tag="mask")
                nc.sync.dma_start(out=mask_t[:], in_=msk[bsl, :, 0])
                mask_ts[bt] = mask_t
                m_all = work.tile([P, N * D], BF16, tag="mnat")
                nc.sync.dma_start(
                    out=m_all.rearrange("p (n d) -> p n d", n=N), in_=mbf[bsl]
                )
                m_alls[bt] = m_all

            a_alls = {}
            c_alls = {}

            def matmuls(bt):
                """a/c chains + PSUM drains (ActE; GPSIMD can't read PSUM)
                for one row-tile. c first so the first pairwise op (which
                needs all c, a_0, a_1) unblocks sooner on the cold start.
                a-chains share a 2-bank PSUM tile per i-pair so one Act copy
                drains two chains (halves the per-copy init cost)."""
                mask_t = mask_ts[bt]
                a_all = work.tile([P, N * D], BF16, tag="a")
                c_all = work.tile([P, N * D], BF16, tag="c")
                a_alls[bt] = a_all
                c_alls[bt] = c_all
                for j in range(N):
                    c_ps = pspc.tile([P, D], F32, tag="cps")
                    for dc in range(DC):
                        nc.tensor.matmul(
                            out=c_ps[:],
                            lhsT=mt[bt, j, dc][:],
                            rhs=wt[1, dc][:],
                            start=(dc == 0),
                            stop=(dc == DC - 1),
                        )
                    # c_j = mask_j * (m_j @ W2): fold the mask into the drain
                    nc.scalar.mul(
                        out=c_all[:, j * D : (j + 1) * D],
                        in_=c_ps[:],
                        mul=mask_t[:, j : j + 1],
                    )
                for i0 in range(0, N, 2):
                    a_ps = pspa.tile([P, 2 * D], F32, tag="aps")
                    for il in range(2):
                        i = i0 + il
                        half = a_ps[:, il * D : (il + 1) * D]
                        for dc in range(DC):
                            nc.tensor.matmul(
                                out=half,
                                lhsT=mt[bt, i, dc][:],
                                rhs=wt[0, dc][:],
                                start=(dc == 0),
                                stop=False,
                            )
                        nc.tensor.matmul(
                            out=half,
                            lhsT=ones_t[:],
                            rhs=bias_t[:],
                            start=False,
                            stop=True,
                        )
                    nc.scalar.copy(
                        out=a_all[:, i0 * D : (i0 + 2) * D], in_=a_ps[:]
                    )

            PREFETCH = 3
            wt, bias_t, ones_t = load_consts()
            for bt in range(min(PREFETCH, nbt)):
                prepass(bt)
            matmuls(0)

            for bt in range(nbt):
                if bt + PREFETCH < nbt:
                    prepass(bt + PREFETCH)
                # pairwise(bt) is emitted BEFORE matmuls(bt+1) so the bt
                # sigmoids aren't queued behind bt+1's PSUM drains in the
                # Act stream; the emission order of the loop tail below is
                # pairwise(bt) work first, then matmuls(bt+1) appended by
                # the next statement group.
                bsl = slice(bt * P, (bt + 1) * P)
                mask_t = mask_ts[bt]
                m_all = m_alls[bt]
                a_all = a_alls[bt]
                c_all = c_alls[bt]

                # ---- u_j = mask_j * m_j (TensorScalar, 4x mode) ----
                u_all = work.tile([P, N * D], BF16, tag="u")
                for j in range(N):
                    nc.vector.tensor_scalar_mul(
                        out=u_all[:, j * D : (j + 1) * D],
                        in0=m_all[:, j * D : (j + 1) * D],
                        scalar1=mask_t[:, j : j + 1],
                    )
                # ---- pairwise sigmoid gating, two i per instruction,
                # diagonal included (cheaper than splitting) ----
                for i0 in range(0, N, 2):
                    a_b = (
                        a_all[:, i0 * D : (i0 + 2) * D]
                        .rearrange("p (i f) -> p i f", i=2)
                        .rearrange("p i (j f) -> p i j f", j=1)
                        .broadcast_to([P, 2, N, D])
                    )
                    c_b = (
                        c_all.rearrange("p (i f) -> p i f", i=1)
                        .broadcast_to([P, 2, N * D])
                        .rearrange("p i (j f) -> p i j f", j=N)
                    )
                    t_all = pairp.tile([P, 2 * N * D], BF16, tag="t")
                    nc.vector.tensor_tensor(
                        out=t_all.rearrange("p (i j f) -> p i j f", i=2, j=N),
                        in0=a_b,
                        in1=c_b,
                        op=_ADD,
                    )
                    g_all = pairp.tile([P, 2 * N * D], BF16, tag="g")
                    nc.scalar.activation(out=g_all[:], in_=t_all[:], func=_SIGMOID)
                    u_b = (
                        u_all.rearrange("p (i f) -> p i f", i=1)
                        .broadcast_to([P, 2, N * D])
                    )
                    p_all = pairp.tile([P, 2 * N * D], BF16, tag="pp")
                    nc.vector.tensor_tensor(
                        out=p_all.rearrange("p (i f) -> p i f", i=2),
                        in0=g_all.rearrange("p (i f) -> p i f", i=2),
                        in1=u_b,
                        op=_MULT,
                    )
                    for il in range(2):
                        i = i0 + il
                        pv = p_all[:, il * N * D : (il + 1) * N * D].rearrange(
                            "p (j f) -> p j f", j=N
                        )
                        s0, s1, rem = _PAIRS[i]
                        s = accp.tile([P, 2 * D], BF16, tag="s")
                        nc.vector.tensor_tensor(
                            out=s.rearrange("p (j f) -> p j f", j=2),
                            in0=pv[:, s0, :],
                            in1=pv[:, s1, :],
                            op=_ADD,
                        )
                        s2 = accp.tile([P, D], BF16, tag="s2")
                        nc.vector.tensor_add(out=s2[:], in0=s[:, :D], in1=s[:, D:])
                        ctx_t = outp.tile([P, D], CTX_DT, tag="ctx")
                        nc.vector.tensor_add(
                            out=ctx_t[:], in0=s2[:], in1=pv[:, rem, :]
                        )
                        nc.scalar.dma_start(out=out[bsl, i, :], in_=ctx_t[:])

                if bt + 1 < nbt:
                    matmuls(bt + 1)
    if split_waits:
        _split_excess_waits(nc)
    return nc


def get_nc(bloc=BLOC, **kw):
    key = (bloc, tuple(sorted(kw.items())))
    if key not in _nc_cache:
        _nc_cache[key] = build(bloc, **kw)
    return _nc_cache[key]


last_results = None


def kernel(**inputs):
    global last_results
    memory = np.ascontiguousarray(inputs["memory"], dtype=np.float32)
    mask = np.ascontiguousarray(inputs["mask"], dtype=np.float32)
    W = np.ascontiguousarray(inputs["W"], dtype=np.float32)
    b = np.ascontiguousarray(inputs["b"], dtype=np.float32)

    nc = get_nc()
    in_maps = [
        {
            "memory": memory[c * BLOC : (c + 1) * BLOC],
            "mask": mask[c * BLOC : (c + 1) * BLOC],
            "W": W,
            "b": b,
        }
        for c in range(NCORES)
    ]
    res = run_bass_kernel_spmd(nc, in_maps, list(range(NCORES)))
    last_results = res
    out = np.concatenate(
        [np.asarray(res.results[c]["context"]) for c in range(NCORES)], axis=0
    )
    return out.astype(np.float32, copy=False)
